# revision 1
# baseline (speedup 1.0000x reference)
"""Causal self-attention with RoPE on 8 Trainium2 NeuronCores.

Sharding: tensor-parallel over heads (4 heads/core) x data-parallel over
batch (2 batches), 8 cores total.  Each core computes QKV projections for
its 4 heads from x[b].T, applies RoPE, runs causal attention, and produces
a partial output projection (row-parallel Wo); the host sums the 4 partials
per batch.

Per-core dataflow (all matmuls bf16 with fp32 PSUM accumulation):
  phase A: qT/kT = Wq_g @ xT FIRST (k-outer accumulation so the PE consumes
           xt chunks in DMA-arrival order -> no startup stall), RoPE rotates
           each head in place right after its projection (DVE/GpSimd work
           hides under subsequent matmuls), THEN v = x @ Wv_g.T last so the
           rope pipeline fully drains during V's 68us of pure PE work and
           the attention phase starts with idle vector engines.
  phase B: per 512-query block: scores TRANSPOSED (k-major: lhsT=kT chunk,
           rhs=qT block) for ALL heads first, exp on ACT (no max-sub;
           scores bounded), causal mask on diagonal chunks, row-sums via a
           ones-vector matmul packed 4-heads-per-PSUM-bank (partitions
           0/32/64/96), normalization folded into the attnT copy-out.
  phase C: partial out = attnT.T @ (Wo.T rows for this group), staged
           through per-m-tile [128, 2048] bf16 tiles (bigger DMA packets,
           half the bytes of fp32; host accumulates partials in fp32).
           Final m-tiles' DMAs are split by row-quarters across queues so
           the kernel doesn't end on one long serial DMA drain.
"""

import sys

sys.path.insert(0, "/opt/trn_rl_repo")

import numpy as np
import ml_dtypes

import concourse.bass as bass
import concourse.mybir as mybir
import concourse.tile as tile
from concourse import bacc
from concourse.bass_utils import run_bass_kernel_spmd

B, C, D, H = 2, 2048, 2048, 16
HD = D // H            # 128 head dim
NCORE = 8
HPC = 4                # heads per core
GW = HPC * HD          # 512: per-core projection width
NKC = D // 128         # 16 contraction chunks
NMT = C // 128         # 16 query m-tiles
NBLK = C // 512        # 4 query blocks
SCALE = 1.0 / np.sqrt(HD)

bf16 = ml_dtypes.bfloat16
BF = mybir.dt.bfloat16
F32 = mybir.dt.float32

TRACE = False
TMPDIR = None
LAST = {}

_nc_cache = []


def _build_nc():
    nc = bacc.Bacc()

    xt_d = nc.declare_dram_parameter("xt", [D, C], BF, isOutput=False)
    wq_d = nc.declare_dram_parameter("wq", [D, GW], BF, isOutput=False)
    wk_d = nc.declare_dram_parameter("wk", [D, GW], BF, isOutput=False)
    wv_d = nc.declare_dram_parameter("wv", [D, GW], BF, isOutput=False)
    wo_d = nc.declare_dram_parameter("wo", [GW, D], BF, isOutput=False)
    cs_d = nc.declare_dram_parameter("cs", [128, C], BF, isOutput=False)
    sn_d = nc.declare_dram_parameter("sn", [128, C], BF, isOutput=False)
    mskT_d = nc.declare_dram_parameter("mskT", [128, 4 * 512], BF,
                                       isOutput=False)
    ones_d = nc.declare_dram_parameter("ones", [128, 1], BF, isOutput=False)
    out_d = nc.declare_dram_parameter("out", [C, D], BF, isOutput=True)

    with tile.TileContext(nc) as tc:
        with tc.tile_pool(name="consts", bufs=1) as cpool, \
             tc.tile_pool(name="vpool", bufs=1) as vpool, \
             tc.tile_pool(name="qkraw", bufs=1) as qkpool, \
             tc.tile_pool(name="rtmp", bufs=8) as rtmp:

            cs_t = cpool.tile([128, C], BF, name="cs_t")
            sn_t = cpool.tile([128, C], BF, name="sn_t")
            mskT_t = cpool.tile([128, 4 * 512], BF, name="mskT_t")
            ones_t = cpool.tile([128, 1], BF, name="ones_t")

            v_sb = [vpool.tile([128, GW], BF, name=f"v{c}") for c in range(NMT)]
            qraw = [qkpool.tile([128, C], BF, name=f"qr{h}") for h in range(HPC)]
            kraw = [qkpool.tile([128, C], BF, name=f"kr{h}") for h in range(HPC)]

            with tc.tile_pool(name="xtp", bufs=1) as xtp, \
                 tc.tile_pool(name="wqk", bufs=1) as wqk, \
                 tc.tile_pool(name="pap", bufs=8, space="PSUM") as pap:
                xt, wq_sb, wk_sb, wv_sb = [], [], [], []
                # k-interleaved: QK's k-outer accumulation consumes chunk k
                # right as it lands, so compute starts ~0.8MB into the stream
                for k in range(NKC):
                    ks = slice(128 * k, 128 * (k + 1))
                    t = xtp.tile([128, C], BF, name=f"xt{k}")
                    tq = wqk.tile([128, GW], BF, name=f"wq{k}")
                    tk = wqk.tile([128, GW], BF, name=f"wk{k}")
                    nc.sync.dma_start(t[:], xt_d[ks, :])
                    nc.sync.dma_start(tq[:], wq_d[ks, :])
                    nc.sync.dma_start(tk[:], wk_d[ks, :])
                    xt.append(t)
                    wq_sb.append(tq)
                    wk_sb.append(tk)
                    if k == 0:
                        nc.sync.dma_start(cs_t[:], cs_d[:])
                        nc.sync.dma_start(sn_t[:], sn_d[:])
                for k in range(NKC):
                    ks = slice(128 * k, 128 * (k + 1))
                    tv = wqk.tile([128, GW], BF, name=f"wv{k}")
                    nc.sync.dma_start(tv[:], wv_d[ks, :])
                    wv_sb.append(tv)
                    if k == 0:
                        nc.sync.dma_start(mskT_t[:], mskT_d[:])
                        nc.sync.dma_start(ones_t[:], ones_d[:])

                # ---- QK projections + in-place RoPE per head, FIRST ----
                # q+k paired per head into all 8 PSUM banks, k-outer: the PE
                # consumes each xt chunk for 8 matmuls right as it lands, so
                # the DMA ramp paces compute smoothly.  One ldweights serves
                # 4 matmuls.  The RoPE rotations (DVE/GpSimd) hide under
                # later matmuls.
                for h in range(HPC):
                    hs = slice(128 * h, 128 * (h + 1))
                    pq8 = [pap.tile([128, 512], F32, name=f"pq{n}",
                                    tag="pa") for n in range(8)]
                    for k in range(NKC):
                        for qk, w_sb in ((0, wq_sb), (1, wk_sb)):
                            for n in range(4):
                                nc.tensor.matmul(
                                    pq8[4 * qk + n][:], w_sb[k][:, hs],
                                    xt[k][:, 512 * n:512 * (n + 1)],
                                    start=(k == 0), stop=(k == NKC - 1))
                    for qk, dst in ((0, qraw[h]), (1, kraw[h])):
                        for n in range(4):
                            ns = slice(512 * n, 512 * (n + 1))
                            nc.scalar.copy(dst[:, ns], pq8[4 * qk + n][:])
                        for n in range(4):
                            ns = slice(512 * n, 512 * (n + 1))
                            tmp = rtmp.tile([128, 512], BF, name="tmp",
                                            tag="rt")
                            nc.vector.tensor_copy(tmp[0:64, :],
                                                  dst[64:128, ns])
                            nc.vector.tensor_copy(tmp[64:128, :],
                                                  dst[0:64, ns])
                            m1 = rtmp.tile([128, 512], BF, name="m1", tag="rt")
                            nc.vector.tensor_mul(m1[:], dst[:, ns],
                                                 cs_t[:, ns])
                            m2 = rtmp.tile([128, 512], BF, name="m2", tag="rt")
                            nc.gpsimd.tensor_mul(m2[:], tmp[:], sn_t[:, ns])
                            nc.vector.tensor_add(dst[:, ns], m1[:], m2[:])

                # ---- phase A tail: V projection (pure PE work; the rope
                # pipeline on DVE/GpSimd drains underneath) ----
                for ct in range(NMT):
                    cts = slice(128 * ct, 128 * (ct + 1))
                    pv = pap.tile([128, GW], F32, name="pv", tag="pa")
                    for k in range(NKC):
                        nc.tensor.matmul(
                            pv[:], xt[k][:, cts], wv_sb[k][:],
                            start=(k == 0), stop=(k == NKC - 1))
                    if ct % 2 == 0:
                        nc.scalar.copy(v_sb[ct][:], pv[:])
                    else:
                        nc.vector.tensor_copy(v_sb[ct][:], pv[:])

            # xt + w pools released here; attention pools reuse the space
            with tc.tile_pool(name="ptile", bufs=18) as ptp, \
                 tc.tile_pool(name="pmm", bufs=4, space="PSUM") as pmm, \
                 tc.tile_pool(name="rsps", bufs=2, space="PSUM") as rsps, \
                 tc.tile_pool(name="attnT", bufs=1) as atp, \
                 tc.tile_pool(name="wop", bufs=1) as wop, \
                 tc.tile_pool(name="sums", bufs=4) as sump, \
                 tc.tile_pool(name="rbp", bufs=2) as rbp, \
                 tc.tile_pool(name="outsb", bufs=3) as outp, \
                 tc.tile_pool(name="qsum", bufs=5) as qsp, \
                 tc.tile_pool(name="qsab", bufs=2) as qsabp, \
                 tc.tile_pool(name="pvps", bufs=2, space="PSUM") as pvps:

                attnT = [atp.tile([128, C], BF, name=f"at{h}") for h in range(HPC)]
                wo_sb = []
                for hk in range(HPC):
                    t = wop.tile([128, D], BF, name=f"wo{hk}")
                    nc.sync.dma_start(t[:], wo_d[128 * hk:128 * (hk + 1), :])
                    wo_sb.append(t)

                def outproj(J, last):
                    # delayed one block so its attnT inputs are long-finished
                    dmaeng = (nc.sync, nc.scalar, nc.gpsimd)
                    for m in range(4 * J, 4 * (J + 1)):
                        ms = slice(128 * m, 128 * (m + 1))
                        ot = outp.tile([128, D], BF, name="ot", tag="ot")
                        for n in range(4):
                            ns = slice(512 * n, 512 * (n + 1))
                            po = pmm.tile([128, 512], F32, name="po",
                                          tag="pmm")
                            for hk in range(HPC):
                                nc.tensor.matmul(po[:], attnT[hk][:, ms],
                                                 wo_sb[hk][:, ns],
                                                 start=(hk == 0),
                                                 stop=(hk == HPC - 1))
                            if n == 3:
                                nc.scalar.copy(ot[:, ns], po[:])
                            else:
                                nc.vector.tensor_copy(ot[:, ns], po[:])
                            if last and m == 4 * J + 3 and n % 2 == 1:
                                # final m-tile: ship each half as soon as its
                                # two copies land so the kernel doesn't end
                                # on a full-tile drain
                                hs_ = slice(1024 * (n // 2),
                                            1024 * (n // 2) + 1024)
                                dmaeng[n // 2].dma_start(out_d[ms, hs_],
                                                         ot[:, hs_])
                        # one DMA per m-tile: each dma_start stripes its
                        # packets over all 16 hw DMA engines, so a single
                        # 512KB transfer drains in ~2us
                        if not (last and m == 4 * J + 3):
                            dmaeng[m % 3].dma_start(out_d[ms, :], ot[:])

                # ---- attention, blocks outer so outproj interleaves ----
                qrot, krot = qraw, kraw  # rotated in place during phase A
                for I in range(NBLK):
                    qs = slice(512 * I, 512 * (I + 1))
                    nch = 4 * (I + 1)
                    # scores+exp for ALL heads first: by the time the ones/PV
                    # matmuls need the probabilities, the scalar engine's exp
                    # pipeline has fully drained -- no PE wait on ACT.
                    # probabilities live in [128, 2048] group tiles of 4
                    # chunks each: regular-stride views enable merged
                    # quad-add reductions for the row-sums
                    ptsh = []
                    for h in range(HPC):
                        bigs = [ptp.tile([128, 2048], BF, name="ptg",
                                         tag="ptile") for _ in range(I + 1)]
                        for c in range(nch):
                            ks = slice(128 * c, 128 * (c + 1))
                            j = c - 4 * I
                            w0 = 128 * j if j > 0 else 0
                            lc = 512 * (c % 4)
                            big = bigs[c // 4]
                            psT = pmm.tile([128, 512], F32, name="psT",
                                           tag="pmm")
                            nc.tensor.matmul(psT[:, w0:512], krot[h][:, ks],
                                             qrot[h][:, 512 * I + w0:
                                                      512 * (I + 1)])
                            if w0 > 0:
                                nc.gpsimd.memset(big[:, lc:lc + w0], 0.0)
                            nc.scalar.activation(
                                big[:, lc + w0:lc + 512], psT[:, w0:512],
                                mybir.ActivationFunctionType.Exp,
                                scale=float(SCALE))
                            if j >= 0:
                                nc.vector.tensor_mul(
                                    big[:, lc + w0:lc + w0 + 128],
                                    big[:, lc + w0:lc + w0 + 128],
                                    mskT_t[:, 512 * j + w0:
                                           512 * j + w0 + 128])
                        ptsh.append(bigs)
                    # row-sums: DVE pre-reduces each head's chunk tiles to
                    # <=4 quad-sums (bf16 adds; ~0.3% rowsum error, well in
                    # budget), so the ones-matmul only streams 4 columns'
                    # worth instead of nch -- saves ~50k PE cycles/core.
                    # h0/h1/h2 at partitions 0/32/64 of one bank, h3 in a
                    # second bank; ALL four chains issue before any reader,
                    # so no tile-WAR stalls the PE between heads.
                    rsA = rsps.tile([128, 512], F32, name="rsA", tag="rs")
                    rsB = rsps.tile([128, 512], F32, name="rsB", tag="rs")

                    def rs_row(h):
                        return (rsA[32 * h:32 * h + 1, :] if h < 3
                                else rsB[0:1, :])

                    for h in range(HPC):
                        bigs = ptsh[h]
                        # pq tiles first so pool rotation recycles only the
                        # dead pab transients while pq awaits the PE read
                        qs_h = [qsp.tile([128, 512], BF, name="pq", tag="qs")
                                for _ in range(nch // 4)]
                        for qi, big in enumerate(bigs):
                            # one [128,1024] add folds chunks (0+1, 2+3)
                            b3 = big[:].rearrange("p (two n) -> p two n",
                                                  two=2)
                            pab = qsabp.tile([128, 1024], BF, name="pab",
                                             tag="qsab")
                            p3 = pab[:].rearrange("p (two n) -> p two n",
                                                  two=2)
                            nc.vector.tensor_add(p3[:, :, :],
                                                 b3[:, :, 0:512],
                                                 b3[:, :, 512:1024])
                            nc.vector.tensor_add(qs_h[qi][:],
                                                 pab[:, 0:512],
                                                 pab[:, 512:1024])
                        for i, pq in enumerate(qs_h):
                            nc.tensor.matmul(
                                rs_row(h),
                                ones_t[:, 0:1],
                                pq[:],
                                start=(i == 0),
                                stop=(i == len(qs_h) - 1))
                    for h in range(HPC):
                        hs = slice(128 * h, 128 * (h + 1))
                        bigs = ptsh[h]
                        # reciprocal on sbuf copy, then broadcast: the whole
                        # rb chain overlaps the PV matmul stream
                        rec = sump.tile([1, 512], F32, name="rec", tag="sm")
                        nc.vector.tensor_copy(rec[:], rs_row(h))
                        nc.vector.reciprocal_approx_fast(out=rec[:],
                                                         in_=rec[:])
                        rb = rbp.tile([128, 512], F32, name="rb", tag="rb")
                        nc.gpsimd.partition_broadcast(rb[:], rec[:])
                        pvp = pvps.tile([128, 512], F32, name="pvp", tag="pv")
                        for c in range(nch):
                            nc.tensor.matmul(pvp[:],
                                             v_sb[c][:, hs],
                                             bigs[c // 4][:, 512 * (c % 4):
                                                          512 * (c % 4) + 512],
                                             start=(c == 0),
                                             stop=(c == nch - 1))
                        nc.vector.tensor_mul(attnT[h][:, qs], pvp[:], rb[:])

                    if I > 0:
                        outproj(I - 1, last=False)
                    if I == NBLK - 1:
                        outproj(I, last=True)

    nc.compile()
    return nc


def _get_nc():
    if not _nc_cache:
        _nc_cache.append(_build_nc())
    return _nc_cache[0]


def _prep_inputs(x, freqs_cos, freqs_sin, Wq, Wk, Wv, Wo):
    # de-interleave permutation within each head's 128 output dims
    perm = np.concatenate([np.arange(0, HD, 2), np.arange(1, HD, 2)])

    cosT = np.ascontiguousarray(freqs_cos.T)  # [64, C]
    sinT = np.ascontiguousarray(freqs_sin.T)
    cs = np.concatenate([cosT, cosT], axis=0).astype(bf16)
    sn = np.concatenate([-sinT, sinT], axis=0).astype(bf16)

    # transposed causal masks for diagonal chunks: chunk c = 4I + j covers
    # keys 128c+p, queries 512I+cc; allowed iff cc >= 128j + p
    p = np.arange(128)[:, None]
    cc = np.arange(512)[None, :]
    mskT = np.concatenate(
        [(cc >= 128 * j + p) for j in range(4)], axis=1).astype(bf16)
    ones = np.ones((128, 1), dtype=bf16)

    xts = [np.ascontiguousarray(x[b].T).astype(bf16) for b in range(B)]

    in_maps = []
    for j in range(NCORE):
        b, g = divmod(j, HPC)
        rows = np.concatenate(
            [512 * g + 128 * hl + perm for hl in range(HPC)])
        rows_nop = np.arange(512 * g, 512 * (g + 1))
        in_maps.append({
            "xt": xts[b],
            "wq": np.ascontiguousarray(Wq[rows, :].T).astype(bf16),
            "wk": np.ascontiguousarray(Wk[rows, :].T).astype(bf16),
            "wv": np.ascontiguousarray(Wv[rows_nop, :].T).astype(bf16),
            "wo": np.ascontiguousarray(Wo[:, rows_nop].T).astype(bf16),
            "cs": cs,
            "sn": sn,
            "mskT": mskT,
            "ones": ones,
        })
    return in_maps


def kernel(x, freqs_cos, freqs_sin, Wq, Wk, Wv, Wo):
    x = np.asarray(x, dtype=np.float32)
    freqs_cos = np.asarray(freqs_cos, dtype=np.float32)
    freqs_sin = np.asarray(freqs_sin, dtype=np.float32)
    Wq = np.asarray(Wq, dtype=np.float32)
    Wk = np.asarray(Wk, dtype=np.float32)
    Wv = np.asarray(Wv, dtype=np.float32)
    Wo = np.asarray(Wo, dtype=np.float32)

    nc = _get_nc()
    in_maps = _prep_inputs(x, freqs_cos, freqs_sin, Wq, Wk, Wv, Wo)
    res = run_bass_kernel_spmd(nc, in_maps, list(range(NCORE)), trace=TRACE,
                               tmpdir=TMPDIR)
    LAST["res"] = res

    out = np.empty((B, C, D), dtype=np.float32)
    for b in range(B):
        acc = res.results[HPC * b]["out"].astype(np.float64)
        for g in range(1, HPC):
            acc += res.results[HPC * b + g]["out"].astype(np.float64)
        out[b] = acc.astype(np.float32)
    return out



# revision 2
# speedup vs baseline: 1.0212x; 1.0212x over previous
"""Causal self-attention with RoPE on 8 Trainium2 NeuronCores.

Sharding: tensor-parallel over heads (4 heads/core) x data-parallel over
batch (2 batches), 8 cores total.  Each core computes QKV projections for
its 4 heads from x[b].T, applies RoPE, runs causal attention, and produces
a partial output projection (row-parallel Wo); the host sums the 4 partials
per batch.

Per-core dataflow (all matmuls bf16 with fp32 PSUM accumulation):
  phase A: PE warm-up dummies during the DMA head, DMA issue spread over
           Sync (xt), Scalar (wq/wk) and GpSimd (wv) queues so transfers
           pipeline.  qT/kT = Wq_g @ xT first (k-outer accumulation
           consumes xt chunks in DMA-arrival order), RoPE rotates each
           head in place, then v = x @ Wv_g.T drains the rope pipeline.
  phase B: scores are computed TRANSPOSED (k-major) in [128,1024] 2-bank
           PSUM group tiles (2 key-chunks each, full 512-query width even
           on the diagonal), exp'd by ONE merged ACTIVATE per group (~30%
           less ACT time than per-chunk exp; masking of the diagonal via
           GpSimd memset + small DVE triangular mul AFTER the exp).  The
           Tensor stream interleaves score groups with PV pairs of the
           previous head, rowsum matmuls, and output-projection slices of
           the previous block through an explicit filler queue, so the
           scalar engine's exp latency never back-pressures the PE and no
           engine idles long enough to re-throttle the HAM clock gate.
           Rowsums: DVE folds each head's probs to one [128,512] tile
           (quad-adds + pair tree), a single ones-vector matmul reduces
           partitions, DVE reciprocal reads the PSUM row directly, GpSimd
           broadcasts it, and the attnT copy-out applies normalization.
  phase C: outproj slices are filler units: po = attnT.T @ Wo chunks in a
           shared 4-buf accumulation pool; each [128,2048] bf16 out tile
           ships as two half DMAs (Sync/GpSimd queues) as soon as its
           columns are copied, so the kernel never ends on a long drain.
"""

import sys

sys.path.insert(0, "/opt/trn_rl_repo")

from collections import deque

import numpy as np
import ml_dtypes

import concourse.bass as bass
import concourse.mybir as mybir
import concourse.tile as tile
from concourse import bacc
from concourse.bass_utils import run_bass_kernel_spmd

B, C, D, H = 2, 2048, 2048, 16
HD = D // H            # 128 head dim
NCORE = 8
HPC = 4                # heads per core
GW = HPC * HD          # 512: per-core projection width
NKC = D // 128         # 16 contraction chunks
NMT = C // 128         # 16 query m-tiles
NBLK = C // 512        # 4 query blocks
SCALE = 1.0 / np.sqrt(HD)

bf16 = ml_dtypes.bfloat16
BF = mybir.dt.bfloat16
F32 = mybir.dt.float32

TRACE = False
TMPDIR = None
LAST = {}

_nc_cache = []


def _build_nc():
    nc = bacc.Bacc()

    xt_d = nc.declare_dram_parameter("xt", [D, C], BF, isOutput=False)
    wq_d = nc.declare_dram_parameter("wq", [D, GW], BF, isOutput=False)
    wk_d = nc.declare_dram_parameter("wk", [D, GW], BF, isOutput=False)
    wv_d = nc.declare_dram_parameter("wv", [D, GW], BF, isOutput=False)
    wo_d = nc.declare_dram_parameter("wo", [GW, D], BF, isOutput=False)
    cs_d = nc.declare_dram_parameter("cs", [128, C], BF, isOutput=False)
    sn_d = nc.declare_dram_parameter("sn", [128, C], BF, isOutput=False)
    mskT_d = nc.declare_dram_parameter("mskT", [128, 4 * 512], BF,
                                       isOutput=False)
    ones_d = nc.declare_dram_parameter("ones", [128, 1], BF, isOutput=False)
    out_d = nc.declare_dram_parameter("out", [C, D], BF, isOutput=True)

    with tile.TileContext(nc) as tc:
        with tc.tile_pool(name="consts", bufs=1) as cpool, \
             tc.tile_pool(name="vpool", bufs=1) as vpool, \
             tc.tile_pool(name="qkraw", bufs=1) as qkpool, \
             tc.tile_pool(name="rtmp", bufs=8) as rtmp:

            cs_t = cpool.tile([128, C], BF, name="cs_t")
            sn_t = cpool.tile([128, C], BF, name="sn_t")
            mskT_t = cpool.tile([128, 4 * 512], BF, name="mskT_t")
            ones_t = cpool.tile([128, 1], BF, name="ones_t")

            v_sb = [vpool.tile([128, GW], BF, name=f"v{c}") for c in range(NMT)]
            qraw = [qkpool.tile([128, C], BF, name=f"qr{h}") for h in range(HPC)]
            kraw = [qkpool.tile([128, C], BF, name=f"kr{h}") for h in range(HPC)]

            with tc.tile_pool(name="xtp", bufs=1) as xtp, \
                 tc.tile_pool(name="wqk", bufs=1) as wqk, \
                 tc.tile_pool(name="pap", bufs=8, space="PSUM") as pap:

                # PE warm-up: dummy matmuls on a zeroed tile issued before
                # any data dependency -- they run during the DMA head so
                # the HAM clock gate is (nearly) released when real
                # matmuls start, instead of paying the 1.2 GHz ramp on
                # real work.
                wsb = xtp.tile([128, 512], BF, name="warm")
                nc.gpsimd.memset(wsb[:], 0.0)
                wps = pap.tile([128, 512], F32, name="wps", tag="pa")
                for _ in range(6):
                    nc.tensor.matmul(wps[:], wsb[:, 0:128], wsb[:],
                                     start=True, stop=True)

                # DMA issue split across queues: xt on Sync, wq/wk on
                # Scalar, wv (+phase-B consts) on GpSimd -- the first
                # matmul's inputs (wq0 + xt0 first half) land ~2.5us
                # after issue instead of serializing behind one queue.
                xt, wq_sb, wk_sb, wv_sb = [], [], [], []
                for k in range(NKC):
                    ks = slice(128 * k, 128 * (k + 1))
                    t = xtp.tile([128, C], BF, name=f"xt{k}")
                    if k == 0:
                        nc.sync.dma_start(t[:, 0:1024], xt_d[ks, 0:1024])
                        nc.sync.dma_start(t[:, 1024:2048],
                                          xt_d[ks, 1024:2048])
                    else:
                        nc.sync.dma_start(t[:], xt_d[ks, :])
                    tq = wqk.tile([128, GW], BF, name=f"wq{k}")
                    tk = wqk.tile([128, GW], BF, name=f"wk{k}")
                    nc.scalar.dma_start(tq[:], wq_d[ks, :])
                    nc.scalar.dma_start(tk[:], wk_d[ks, :])
                    xt.append(t)
                    wq_sb.append(tq)
                    wk_sb.append(tk)
                nc.sync.dma_start(cs_t[:], cs_d[:])
                nc.sync.dma_start(sn_t[:], sn_d[:])
                for k in range(NKC):
                    ks = slice(128 * k, 128 * (k + 1))
                    tv = wqk.tile([128, GW], BF, name=f"wv{k}")
                    nc.gpsimd.dma_start(tv[:], wv_d[ks, :])
                    wv_sb.append(tv)
                nc.gpsimd.dma_start(mskT_t[:], mskT_d[:])
                nc.gpsimd.dma_start(ones_t[:], ones_d[:])

                # ---- QK projections + in-place RoPE per head, FIRST ----
                # q+k paired per head into all 8 PSUM banks, k-outer: the PE
                # consumes each xt chunk for 8 matmuls right as it lands, so
                # the DMA ramp paces compute smoothly.  One ldweights serves
                # 4 matmuls.  The RoPE rotations (DVE/GpSimd) hide under
                # later matmuls.
                for h in range(HPC):
                    hs = slice(128 * h, 128 * (h + 1))
                    pq8 = [pap.tile([128, 512], F32, name=f"pq{n}",
                                    tag="pa") for n in range(8)]
                    for k in range(NKC):
                        for qk, w_sb in ((0, wq_sb), (1, wk_sb)):
                            for n in range(4):
                                nc.tensor.matmul(
                                    pq8[4 * qk + n][:], w_sb[k][:, hs],
                                    xt[k][:, 512 * n:512 * (n + 1)],
                                    start=(k == 0), stop=(k == NKC - 1))
                    for qk, dst in ((0, qraw[h]), (1, kraw[h])):
                        for n in range(4):
                            ns = slice(512 * n, 512 * (n + 1))
                            nc.scalar.copy(dst[:, ns], pq8[4 * qk + n][:])
                        for n in range(4):
                            ns = slice(512 * n, 512 * (n + 1))
                            tmp = rtmp.tile([128, 512], BF, name="tmp",
                                            tag="rt")
                            nc.vector.tensor_copy(tmp[0:64, :],
                                                  dst[64:128, ns])
                            nc.vector.tensor_copy(tmp[64:128, :],
                                                  dst[0:64, ns])
                            m1 = rtmp.tile([128, 512], BF, name="m1", tag="rt")
                            nc.vector.tensor_mul(m1[:], dst[:, ns],
                                                 cs_t[:, ns])
                            m2 = rtmp.tile([128, 512], BF, name="m2", tag="rt")
                            nc.gpsimd.tensor_mul(m2[:], tmp[:], sn_t[:, ns])
                            nc.vector.tensor_add(dst[:, ns], m1[:], m2[:])

                # ---- phase A tail: V projection (pure PE work; the rope
                # pipeline on DVE/GpSimd drains underneath) ----
                for ct in range(NMT):
                    cts = slice(128 * ct, 128 * (ct + 1))
                    pv = pap.tile([128, GW], F32, name="pv", tag="pa")
                    for k in range(NKC):
                        nc.tensor.matmul(
                            pv[:], xt[k][:, cts], wv_sb[k][:],
                            start=(k == 0), stop=(k == NKC - 1))
                    if ct % 2 == 0:
                        nc.scalar.copy(v_sb[ct][:], pv[:])
                    else:
                        nc.vector.tensor_copy(v_sb[ct][:], pv[:])

            # xt + w pools released here; attention pools reuse the space
            with tc.tile_pool(name="ptile", bufs=17) as ptp, \
                 tc.tile_pool(name="sg", bufs=2, space="PSUM") as sgp, \
                 tc.tile_pool(name="acc", bufs=4, space="PSUM") as accp, \
                 tc.tile_pool(name="attnT", bufs=1) as atp, \
                 tc.tile_pool(name="wop", bufs=1) as wop, \
                 tc.tile_pool(name="recp", bufs=4) as recp, \
                 tc.tile_pool(name="rbp", bufs=3) as rbp, \
                 tc.tile_pool(name="outsb", bufs=3) as outp, \
                 tc.tile_pool(name="qsum", bufs=6) as qsp, \
                 tc.tile_pool(name="qsab", bufs=2) as qsabp:

                attnT = [atp.tile([128, C], BF, name=f"at{h}")
                         for h in range(HPC)]
                wo_sb = []
                for hk in range(HPC):
                    t = wop.tile([128, D], BF, name=f"wo{hk}")
                    nc.sync.dma_start(t[:], wo_d[128 * hk:128 * (hk + 1), :])
                    wo_sb.append(t)

                qrot, krot = qraw, kraw  # rotated in place during phase A

                # ---- filler-queue machinery: the Tensor stream is built
                # as score-group units with ~0.8us of other PE work
                # interleaved after each, popped from two queues:
                #   pe_q: PV pairs / rowsum / attnT units of earlier heads
                #   op_q: output-projection slices of the previous block
                pe_q = deque()
                op_q = deque()
                pv_t, rb_t = {}, {}
                ot_tiles = {}

                def mk_op_unit(m, n):
                    def emit():
                        ms = slice(128 * m, 128 * (m + 1))
                        ns = slice(512 * n, 512 * (n + 1))
                        if n == 0:
                            ot_tiles[m] = outp.tile([128, D], BF, name="ot",
                                                    tag="ot")
                        ot = ot_tiles[m]
                        po = accp.tile([128, 512], F32, name="po", tag="acc")
                        for hk in range(HPC):
                            nc.tensor.matmul(po[:], attnT[hk][:, ms],
                                             wo_sb[hk][:, ns],
                                             start=(hk == 0),
                                             stop=(hk == HPC - 1))
                        if n == 3:
                            nc.scalar.copy(ot[:, ns], po[:])
                        else:
                            nc.vector.tensor_copy(ot[:, ns], po[:])
                        # ship each half as soon as its two copies land
                        if n == 1:
                            nc.sync.dma_start(out_d[ms, 0:1024],
                                              ot[:, 0:1024])
                        elif n == 3:
                            nc.gpsimd.dma_start(out_d[ms, 1024:2048],
                                                ot[:, 1024:2048])
                    return emit

                def mk_pv_pair(h, bigs, p, nch):
                    def emit():
                        if p == 0:
                            pv_t[h] = accp.tile([128, 512], F32, name="pv",
                                                tag="acc")
                        pvp = pv_t[h]
                        hsl = slice(128 * h, 128 * (h + 1))
                        for c in (2 * p, 2 * p + 1):
                            nc.tensor.matmul(
                                pvp[:], v_sb[c][:, hsl],
                                bigs[c // 4][:, 512 * (c % 4):
                                             512 * (c % 4) + 512],
                                start=(c == 0), stop=(c == nch - 1))
                    return emit

                def mk_rs(h, qstot):
                    def emit():
                        rs = accp.tile([128, 512], F32, name="rs", tag="acc")
                        nc.tensor.matmul(rs[0:1, :], ones_t[:, 0:1],
                                         qstot[:], start=True, stop=True)
                        rec = recp.tile([1, 512], F32, name="rec", tag="rec")
                        nc.vector.reciprocal_approx_fast(out=rec[:],
                                                         in_=rs[0:1, :])
                        rb = rbp.tile([128, 512], F32, name="rb", tag="rb")
                        nc.gpsimd.partition_broadcast(rb[:], rec[:])
                        rb_t[h] = rb
                    return emit

                def mk_attnT(h, I):
                    def emit():
                        qs = slice(512 * I, 512 * (I + 1))
                        nc.vector.tensor_mul(attnT[h][:, qs], pv_t[h][:],
                                             rb_t[h][:])
                        if h == HPC - 1:
                            for m in range(4 * I, 4 * I + 4):
                                for n in range(4):
                                    op_q.append((852, mk_op_unit(m, n)))
                    return emit

                def emit_fillers(budget_ns):
                    spent = 0
                    while spent < budget_ns:
                        if pe_q:
                            cost, emit = pe_q.popleft()
                        elif op_q:
                            cost, emit = op_q.popleft()
                        else:
                            return
                        emit()
                        spent += cost

                for I in range(NBLK):
                    nch, ngrp = 4 * (I + 1), 2 * (I + 1)
                    qs = slice(512 * I, 512 * (I + 1))
                    for h in range(HPC):
                        bigs = [ptp.tile([128, C], BF, name="ptg",
                                         tag="ptile") for _ in range(I + 1)]
                        for g in range(ngrp):
                            # score group: 2 key-chunks, full query width,
                            # one merged exp over the 2-bank PSUM tile
                            sg = sgp.tile([128, 1024], F32, name="sg",
                                          tag="sg")
                            for i, c in enumerate((2 * g, 2 * g + 1)):
                                nc.tensor.matmul(
                                    sg[:, 512 * i:512 * (i + 1)],
                                    krot[h][:, 128 * c:128 * (c + 1)],
                                    qrot[h][:, qs],
                                    start=True, stop=True)
                            big = bigs[g // 2]
                            lcg = 1024 * (g % 2)
                            nc.scalar.activation(
                                big[:, lcg:lcg + 1024], sg[:],
                                mybir.ActivationFunctionType.Exp,
                                scale=float(SCALE))
                            for c in (2 * g, 2 * g + 1):
                                j = c - 4 * I
                                if j >= 0:
                                    lc = 512 * (c % 4)
                                    w0 = 128 * j
                                    if w0 > 0:
                                        nc.gpsimd.memset(big[:, lc:lc + w0],
                                                         0.0)
                                    nc.vector.tensor_mul(
                                        big[:, lc + w0:lc + w0 + 128],
                                        big[:, lc + w0:lc + w0 + 128],
                                        mskT_t[:, 512 * j + w0:
                                               512 * j + w0 + 128])
                            emit_fillers(800)
                        # rowsums on DVE: fold each big to [128,512], then
                        # pair-tree to one tile per head
                        qtiles = []
                        for big in bigs:
                            b3 = big[:].rearrange("p (two n) -> p two n",
                                                  two=2)
                            pab = qsabp.tile([128, 1024], BF, name="pab",
                                             tag="qsab")
                            p3 = pab[:].rearrange("p (two n) -> p two n",
                                                  two=2)
                            nc.vector.tensor_add(p3[:, :, :],
                                                 b3[:, :, 0:512],
                                                 b3[:, :, 512:1024])
                            q = qsp.tile([128, 512], BF, name="pq", tag="qs")
                            nc.vector.tensor_add(q[:], pab[:, 0:512],
                                                 pab[:, 512:1024])
                            qtiles.append(q)
                        while len(qtiles) > 1:
                            a = qtiles.pop(0)
                            b = qtiles.pop(0)
                            t = qsp.tile([128, 512], BF, name="pq2",
                                         tag="qs")
                            nc.vector.tensor_add(t[:], a[:], b[:])
                            qtiles.append(t)
                        # queue this head's PE work as fillers for the
                        # next head's score stream
                        for p in range(ngrp):
                            pe_q.append((426, mk_pv_pair(h, bigs, p, nch)))
                        pe_q.append((300, mk_rs(h, qtiles[0])))
                        pe_q.append((100, mk_attnT(h, I)))

                # drain all remaining PV/rowsum/outproj work
                emit_fillers(1 << 60)
                emit_fillers(1 << 60)

    nc.compile()
    return nc


def _get_nc():
    if not _nc_cache:
        _nc_cache.append(_build_nc())
    return _nc_cache[0]


def _prep_inputs(x, freqs_cos, freqs_sin, Wq, Wk, Wv, Wo):
    # de-interleave permutation within each head's 128 output dims
    perm = np.concatenate([np.arange(0, HD, 2), np.arange(1, HD, 2)])

    cosT = np.ascontiguousarray(freqs_cos.T)  # [64, C]
    sinT = np.ascontiguousarray(freqs_sin.T)
    cs = np.concatenate([cosT, cosT], axis=0).astype(bf16)
    sn = np.concatenate([-sinT, sinT], axis=0).astype(bf16)

    # transposed causal masks for diagonal chunks: chunk c = 4I + j covers
    # keys 128c+p, queries 512I+cc; allowed iff cc >= 128j + p
    p = np.arange(128)[:, None]
    cc = np.arange(512)[None, :]
    mskT = np.concatenate(
        [(cc >= 128 * j + p) for j in range(4)], axis=1).astype(bf16)
    ones = np.ones((128, 1), dtype=bf16)

    xts = [np.ascontiguousarray(x[b].T).astype(bf16) for b in range(B)]

    in_maps = []
    for j in range(NCORE):
        b, g = divmod(j, HPC)
        rows = np.concatenate(
            [512 * g + 128 * hl + perm for hl in range(HPC)])
        rows_nop = np.arange(512 * g, 512 * (g + 1))
        in_maps.append({
            "xt": xts[b],
            "wq": np.ascontiguousarray(Wq[rows, :].T).astype(bf16),
            "wk": np.ascontiguousarray(Wk[rows, :].T).astype(bf16),
            "wv": np.ascontiguousarray(Wv[rows_nop, :].T).astype(bf16),
            "wo": np.ascontiguousarray(Wo[:, rows_nop].T).astype(bf16),
            "cs": cs,
            "sn": sn,
            "mskT": mskT,
            "ones": ones,
        })
    return in_maps


def kernel(x, freqs_cos, freqs_sin, Wq, Wk, Wv, Wo):
    x = np.asarray(x, dtype=np.float32)
    freqs_cos = np.asarray(freqs_cos, dtype=np.float32)
    freqs_sin = np.asarray(freqs_sin, dtype=np.float32)
    Wq = np.asarray(Wq, dtype=np.float32)
    Wk = np.asarray(Wk, dtype=np.float32)
    Wv = np.asarray(Wv, dtype=np.float32)
    Wo = np.asarray(Wo, dtype=np.float32)

    nc = _get_nc()
    in_maps = _prep_inputs(x, freqs_cos, freqs_sin, Wq, Wk, Wv, Wo)
    res = run_bass_kernel_spmd(nc, in_maps, list(range(NCORE)), trace=TRACE,
                               tmpdir=TMPDIR)
    LAST["res"] = res

    out = np.empty((B, C, D), dtype=np.float32)
    for b in range(B):
        acc = res.results[HPC * b]["out"].astype(np.float64)
        for g in range(1, HPC):
            acc += res.results[HPC * b + g]["out"].astype(np.float64)
        out[b] = acc.astype(np.float32)
    return out
